# revision 2
# baseline (speedup 1.0000x reference)
"""DRAW-style recurrent VAE kernel for 8 Trainium2 NeuronCores.

Self-contained: takes FULL inputs (as produced by setup_inputs()), returns the
FULL (B, L) output of sigmoid(c_T).

Sharding (model-parallel, NOT data-parallel -- the 200MB of weights do not fit
in one core's SBUF replicated, and TensorE matmul cost is independent of the
batch/M dim so batch sharding saves nothing):
  - core k owns gate rows Gk = [Hk, H+Hk, 2H+Hk, 3H+Hk] (512 of 4096 gates,
    i.e. h-slice Hk = [128k, 128k+128) of both LSTMs) and canvas columns
    Lk = [512k, 512k+512).
  - per step: 3 AllGathers (S^T slices in bf16, h_enc^T slice, h_dec^T slice).
  - x @ (W1+W2).T is constant across timesteps -> computed once in a preamble
    (r_t = [x, x - sigmoid(c_t), h_dec] so W_ih@r = (W1+W2)@x - W2@S + W3@h).
  - matmuls run in float32r (full PE rate); the big -S@W2 term runs in bf16
    (W2 resident bf16 halves SBUF + AG traffic); exp() computed as
    sigmoid(x)/sigmoid(-x) to stay in the sigmoid+tanh ACT table set.
"""

import numpy as np
import ml_dtypes

import concourse.bass as bass  # noqa: F401
import concourse.mybir as mybir
import concourse.tile as tile
from concourse import bacc
from concourse.bass_utils import run_bass_kernel_spmd
from concourse.masks import make_identity

F32 = mybir.dt.float32
F32R = mybir.dt.float32r
BF16 = mybir.dt.bfloat16
AFT = mybir.ActivationFunctionType

B, L, H, Z, T = 128, 4096, 1024, 256, 16
NCORES = 8
LS = L // NCORES          # 512 canvas cols per core
HS = H // NCORES          # 128 h rows per core
GS = 4 * HS               # 512 gate rows per core
KL = L // 128             # 32 K-tiles over L
KH = H // 128             # 8  K-tiles over H
KZ = Z // 128             # 2  K-tiles over Z


def _load_T(nc, sb_tile, dram_ap, ktiles, n, nchunks):
    """DRAM (ktiles*128, n) row-major -> SBUF tile [128, ktiles*n] where
    free-dim slice [kt*n:(kt+1)*n] holds K-tile kt. Split into nchunks DMAs."""
    per = ktiles // nchunks
    assert per * nchunks == ktiles
    for c in range(nchunks):
        src = dram_ap[c * per * 128:(c + 1) * per * 128, :]
        src = src.rearrange("(t p) n -> p t n", t=per)
        dst = sb_tile[:, c * per * n:(c + 1) * per * n]
        dst = dst.rearrange("p (t n) -> p t n", t=per)
        nc.sync.dma_start(dst, src)


def build(repeats=1, debug=False):
    nc = bacc.Bacc("TRN2", target_bir_lowering=False, debug=False,
                   num_devices=NCORES)

    def inp(name, shape, dt=F32R):
        return nc.dram_tensor(name, list(shape), dt, kind="ExternalInput").ap()

    xT = inp("xT", (L, B))                  # x.T (replicated)
    w12t = inp("w12t", (L, GS))             # (W1+W2)[Gk].T (streamed, preamble)
    w2t = inp("w2t", (L, GS), BF16)         # -W2[Gk].T  (resident, negated!)
    w3t = inp("w3t", (H, GS))
    whht = inp("whht", (H, GS))
    wmst = inp("wmst", (H, GS))             # [W_mu; W_sigma].T (replicated)
    widt = inp("widt", (Z, GS))
    whdt = inp("whdt", (H, GS))
    wwrt = inp("wwrt", (H, LS))             # W_write[Lk].T
    bias_e = inp("bias_e", (B, GS), F32)    # (b_ih_enc+b_hh_enc)[Gk] bcast
    bias_ms = inp("bias_ms", (B, GS), F32)
    bias_d = inp("bias_d", (B, GS), F32)
    bias_w = inp("bias_w", (B, LS), F32)
    c0b = inp("c0b", (B, LS), F32)          # c_0[Lk] broadcast over batch
    h0eT = inp("h0eT", (H, B))              # h_0_enc.T bcast
    h0dT = inp("h0dT", (H, B))
    eps_all = inp("eps", (T, B, Z), F32)

    out = nc.dram_tensor("out", [B, LS], F32, kind="ExternalOutput").ap()
    dbg = {}
    if debug:
        for nm, shape in [("dbg_heT", (H, B)), ("dbg_hdT", (H, B)),
                          ("dbg_ct", (B, LS)), ("dbg_ms", (B, GS)),
                          ("dbg_ge", (B, GS))]:
            dbg[nm] = nc.dram_tensor(nm, list(shape), F32,
                                     kind="ExternalOutput").ap()

    # collective bounce buffers (double-buffered across steps)
    ag = []
    for i in range(2):
        ag.append({
            "s_in": nc.dram_tensor(f"ag_s_in{i}", [LS, B], BF16),
            "s_out": nc.dram_tensor(f"ag_s_out{i}", [L, B], BF16,
                                    addr_space="Shared"),
            "e_in": nc.dram_tensor(f"ag_e_in{i}", [HS, B], F32),
            "e_out": nc.dram_tensor(f"ag_e_out{i}", [H, B], F32,
                                    addr_space="Shared"),
            "d_in": nc.dram_tensor(f"ag_d_in{i}", [HS, B], F32),
            "d_out": nc.dram_tensor(f"ag_d_out{i}", [H, B], F32,
                                    addr_space="Shared"),
        })

    RG = [list(range(NCORES))]

    with tile.TileContext(nc) as tc:
        with tc.tile_pool(name="weights", bufs=1) as wp, \
             tc.tile_pool(name="state", bufs=1) as st, \
             tc.tile_pool(name="mmps", bufs=4, space="PSUM") as ps, \
             tc.tile_pool(name="trps", bufs=3, space="PSUM") as tr, \
             tc.tile_pool(name="epspool", bufs=2) as epp:

            # ---------------- resident weights ----------------
            w2_sb = wp.tile([128, KL * GS], BF16)    # 32KB/part
            _load_T(nc, w2_sb, w2t, KL, GS, 8)
            w3_sb = wp.tile([128, KH * GS], F32R)
            _load_T(nc, w3_sb, w3t, KH, GS, 2)
            whh_sb = wp.tile([128, KH * GS], F32R)
            _load_T(nc, whh_sb, whht, KH, GS, 2)
            wms_sb = wp.tile([128, KH * GS], F32R)
            _load_T(nc, wms_sb, wmst, KH, GS, 2)
            wid_sb = wp.tile([128, KZ * GS], F32R)
            _load_T(nc, wid_sb, widt, KZ, GS, 1)
            whd_sb = wp.tile([128, KH * GS], F32R)
            _load_T(nc, whd_sb, whdt, KH, GS, 2)
            wwr_sb = wp.tile([128, KH * LS], F32R)
            _load_T(nc, wwr_sb, wwrt, KH, LS, 2)

            be_sb = wp.tile([B, GS], F32)
            nc.sync.dma_start(be_sb[:], bias_e)
            bms_sb = wp.tile([B, GS], F32)
            nc.sync.dma_start(bms_sb[:], bias_ms)
            bd_sb = wp.tile([B, GS], F32)
            nc.sync.dma_start(bd_sb[:], bias_d)
            bw_sb = wp.tile([B, LS], F32)
            nc.sync.dma_start(bw_sb[:], bias_w)

            ident = wp.tile([128, 128], F32)
            make_identity(nc, ident[:])

            # persistent state
            c_t = st.tile([B, LS], F32)
            c_enc = st.tile([B, HS], F32)
            c_dec = st.tile([B, HS], F32)
            heT = st.tile([128, KH * B], F32R)   # h_enc.T full (8 slots)
            hdT = st.tile([128, KH * B], F32R)
            sT = st.tile([128, KL * B], BF16)    # S.T full (32 slots)
            const_sb = st.tile([B, GS], F32)

            for rep in range(repeats):
                # -------------- init state --------------
                nc.sync.dma_start(c_t[:], c0b)
                nc.gpsimd.memset(c_enc[:], 0.0)
                nc.gpsimd.memset(c_dec[:], 0.0)
                _load_T(nc, heT, h0eT, KH, B, 2)
                _load_T(nc, hdT, h0dT, KH, B, 2)

                # ---- preamble: const = x@(W1+W2)[Gk].T + bias_e ----
                with tc.tile_pool(name="pre", bufs=3) as pre:
                    cst_ps = ps.tile([B, GS], F32, tag="mm")
                    for c in range(8):
                        xT_sb = pre.tile([128, 4 * B], F32R, tag="xT")
                        src = xT[c * 512:(c + 1) * 512, :]
                        nc.sync.dma_start(
                            xT_sb[:].rearrange("p (t n) -> p t n", t=4),
                            src.rearrange("(t p) n -> p t n", t=4))
                        w12_sb = pre.tile([128, 4 * GS], F32R, tag="w12")
                        src = w12t[c * 512:(c + 1) * 512, :]
                        nc.sync.dma_start(
                            w12_sb[:].rearrange("p (t n) -> p t n", t=4),
                            src.rearrange("(t p) n -> p t n", t=4))
                        for j in range(4):
                            kt = c * 4 + j
                            nc.tensor.matmul(
                                cst_ps[:],
                                xT_sb[:, j * B:(j + 1) * B],
                                w12_sb[:, j * GS:(j + 1) * GS],
                                start=(kt == 0), stop=(kt == KL - 1))
                    nc.vector.tensor_add(const_sb[:], cst_ps[:], be_sb[:])

                # -------------- time loop --------------
                with tc.tile_pool(name="work", bufs=2) as wk:
                    for t in range(T):
                        bufs = ag[t % 2]
                        # (1) enc-gate PSUM opens with Whh (h_encT[t-1] is
                        # available early) -> PE busy while AG(h_dec) lands.
                        ge_ps = ps.tile([B, GS], F32, tag="mm")
                        for kt in range(KH):
                            nc.tensor.matmul(
                                ge_ps[:], heT[:, kt * B:(kt + 1) * B],
                                whh_sb[:, kt * GS:(kt + 1) * GS],
                                start=(kt == 0), stop=False,
                                skip_group_check=True)

                        # (2) canvas update: c_t += h_dec@W_write.T + b_w
                        if t > 0:
                            cd_ps = ps.tile([B, LS], F32, tag="mm")
                            for kt in range(KH):
                                nc.tensor.matmul(
                                    cd_ps[:], hdT[:, kt * B:(kt + 1) * B],
                                    wwr_sb[:, kt * LS:(kt + 1) * LS],
                                    start=(kt == 0), stop=(kt == KH - 1),
                                    skip_group_check=True)
                            nc.vector.tensor_add(c_t[:], c_t[:], cd_ps[:])
                            nc.vector.tensor_add(c_t[:], c_t[:], bw_sb[:])

                        # (3) S = sigmoid(c_t); transpose; AG contribution
                        s_b = wk.tile([B, LS], F32, tag="s_b")
                        nc.scalar.activation(s_b[:], c_t[:], AFT.Sigmoid)
                        s_ctb = wk.tile([128, 4 * B], BF16, tag="s_ctb")
                        for j in range(4):
                            tp = tr.tile([128, 128], F32, tag="tr")
                            nc.tensor.transpose(
                                tp[:], s_b[:, j * 128:(j + 1) * 128], ident[:])
                            nc.vector.tensor_copy(
                                s_ctb[:, j * B:(j + 1) * B], tp[:])
                        nc.sync.dma_start(
                            bufs["s_in"].ap().rearrange("(t p) n -> p t n", t=4),
                            s_ctb[:].rearrange("p (t n) -> p t n", t=4))
                        nc.gpsimd.collective_compute(
                            "AllGather", mybir.AluOpType.bypass,
                            replica_groups=RG,
                            ins=[bufs["s_in"].ap().opt()],
                            outs=[bufs["s_out"].ap().opt()])

                        # (4) W3 term (h_decT): fills AG(S) latency
                        for kt in range(KH):
                            nc.tensor.matmul(
                                ge_ps[:], hdT[:, kt * B:(kt + 1) * B],
                                w3_sb[:, kt * GS:(kt + 1) * GS],
                                start=False, stop=False, skip_group_check=True)

                        # (5) load S.T full; -S@W2 accumulation (32 MMs)
                        for c in range(8):
                            src = bufs["s_out"].ap()[c * 512:(c + 1) * 512, :]
                            nc.sync.dma_start(
                                sT[:, c * 4 * B:(c + 1) * 4 * B]
                                .rearrange("p (t n) -> p t n", t=4),
                                src.rearrange("(t p) n -> p t n", t=4))
                        for kt in range(KL):
                            nc.tensor.matmul(
                                ge_ps[:], sT[:, kt * B:(kt + 1) * B],
                                w2_sb[:, kt * GS:(kt + 1) * GS],
                                start=False, stop=(kt == KL - 1),
                                skip_group_check=True)

                        # (6) encoder LSTM pointwise
                        ge = wk.tile([B, GS], F32, tag="ge")
                        nc.vector.tensor_add(ge[:], ge_ps[:], const_sb[:])
                        if debug and t == 0 and rep == 0:
                            nc.sync.dma_start(dbg["dbg_ge"], ge[:])
                        sif = wk.tile([B, 256], F32, tag="sif")
                        nc.scalar.activation(sif[:], ge[:, 0:256], AFT.Sigmoid)
                        tg = wk.tile([B, HS], F32, tag="tg")
                        nc.scalar.activation(tg[:], ge[:, 256:384], AFT.Tanh)
                        so = wk.tile([B, HS], F32, tag="so")
                        nc.scalar.activation(so[:], ge[:, 384:512], AFT.Sigmoid)
                        t2 = wk.tile([B, HS], F32, tag="t2")
                        nc.vector.tensor_mul(t2[:], sif[:, 0:128], tg[:])
                        nc.vector.tensor_mul(c_enc[:], c_enc[:],
                                             sif[:, 128:256])
                        nc.vector.tensor_add(c_enc[:], c_enc[:], t2[:])
                        tc_ = wk.tile([B, HS], F32, tag="tc_")
                        nc.scalar.activation(tc_[:], c_enc[:], AFT.Tanh)
                        h_e = wk.tile([B, HS], F32, tag="h_e")
                        nc.vector.tensor_mul(h_e[:], so[:], tc_[:])

                        # (7) transpose h_enc; AG
                        tp = tr.tile([128, 128], F32, tag="tr")
                        nc.tensor.transpose(tp[:], h_e[:], ident[:])
                        he_ctb = wk.tile([128, B], F32R, tag="he_ctb")
                        nc.vector.tensor_copy(he_ctb[:], tp[:])
                        nc.sync.dma_start(bufs["e_in"].ap(),
                                          he_ctb[:].bitcast(F32))
                        nc.gpsimd.collective_compute(
                            "AllGather", mybir.AluOpType.bypass,
                            replica_groups=RG,
                            ins=[bufs["e_in"].ap().opt()],
                            outs=[bufs["e_out"].ap().opt()])

                        # (8) dec-gate PSUM opens with Whd (h_decT[t-1]):
                        # fills AG(h_enc) latency
                        gd_ps = ps.tile([B, GS], F32, tag="mm")
                        for kt in range(KH):
                            nc.tensor.matmul(
                                gd_ps[:], hdT[:, kt * B:(kt + 1) * B],
                                whd_sb[:, kt * GS:(kt + 1) * GS],
                                start=(kt == 0), stop=False,
                                skip_group_check=True)

                        # (9) reload h_encT full; musig
                        for c in range(2):
                            src = bufs["e_out"].ap()[c * 512:(c + 1) * 512, :]
                            nc.sync.dma_start(
                                heT[:, c * 4 * B:(c + 1) * 4 * B]
                                .rearrange("p (t n) -> p t n", t=4),
                                src.rearrange("(t p) n -> p t n", t=4)
                                .bitcast(F32R))
                        ms_ps = ps.tile([B, GS], F32, tag="mm")
                        for kt in range(KH):
                            nc.tensor.matmul(
                                ms_ps[:], heT[:, kt * B:(kt + 1) * B],
                                wms_sb[:, kt * GS:(kt + 1) * GS],
                                start=(kt == 0), stop=(kt == KH - 1),
                                skip_group_check=True)
                        ms = wk.tile([B, GS], F32, tag="ms")
                        nc.vector.tensor_add(ms[:], ms_ps[:], bms_sb[:])
                        if debug and t == T - 1 and rep == 0:
                            nc.sync.dma_start(dbg["dbg_ms"], ms[:])

                        # (10) z = eps*exp(logsig) + mu; exp = sig(x)/sig(-x)
                        eps_sb = epp.tile([B, Z], F32, tag="eps")
                        nc.sync.dma_start(eps_sb[:], eps_all[t])
                        spz = wk.tile([B, Z], F32, tag="spz")
                        nc.scalar.activation(spz[:], ms[:, 256:512],
                                             AFT.Sigmoid)
                        snz = wk.tile([B, Z], F32, tag="snz")
                        nc.scalar.activation(snz[:], ms[:, 256:512],
                                             AFT.Sigmoid, scale=-1.0)
                        rnz = wk.tile([B, Z], F32, tag="rnz")
                        nc.vector.reciprocal(rnz[:], snz[:])
                        nc.vector.tensor_mul(rnz[:], spz[:], rnz[:])
                        nc.vector.tensor_mul(rnz[:], eps_sb[:], rnz[:])
                        z = wk.tile([B, Z], F32, tag="z")
                        nc.vector.tensor_add(z[:], rnz[:], ms[:, 0:256])

                        # (11) z.T (2 tiles); finish decoder gates with Wid
                        zT = wk.tile([128, KZ * B], F32R, tag="zT")
                        for j in range(KZ):
                            tp = tr.tile([128, 128], F32, tag="tr")
                            nc.tensor.transpose(
                                tp[:], z[:, j * 128:(j + 1) * 128], ident[:])
                            nc.vector.tensor_copy(
                                zT[:, j * B:(j + 1) * B], tp[:])
                        for kt in range(KZ):
                            nc.tensor.matmul(
                                gd_ps[:], zT[:, kt * B:(kt + 1) * B],
                                wid_sb[:, kt * GS:(kt + 1) * GS],
                                start=False, stop=(kt == KZ - 1),
                                skip_group_check=True)

                        # (12) decoder LSTM pointwise
                        gd = wk.tile([B, GS], F32, tag="gd")
                        nc.vector.tensor_add(gd[:], gd_ps[:], bd_sb[:])
                        sifd = wk.tile([B, 256], F32, tag="sifd")
                        nc.scalar.activation(sifd[:], gd[:, 0:256],
                                             AFT.Sigmoid)
                        tgd = wk.tile([B, HS], F32, tag="tgd")
                        nc.scalar.activation(tgd[:], gd[:, 256:384], AFT.Tanh)
                        sod = wk.tile([B, HS], F32, tag="sod")
                        nc.scalar.activation(sod[:], gd[:, 384:512],
                                             AFT.Sigmoid)
                        t2d = wk.tile([B, HS], F32, tag="t2d")
                        nc.vector.tensor_mul(t2d[:], sifd[:, 0:128], tgd[:])
                        nc.vector.tensor_mul(c_dec[:], c_dec[:],
                                             sifd[:, 128:256])
                        nc.vector.tensor_add(c_dec[:], c_dec[:], t2d[:])
                        tcd = wk.tile([B, HS], F32, tag="tcd")
                        nc.scalar.activation(tcd[:], c_dec[:], AFT.Tanh)
                        h_d = wk.tile([B, HS], F32, tag="h_d")
                        nc.vector.tensor_mul(h_d[:], sod[:], tcd[:])

                        # (13) transpose h_dec; AG; reload h_decT full
                        tp = tr.tile([128, 128], F32, tag="tr")
                        nc.tensor.transpose(tp[:], h_d[:], ident[:])
                        hd_ctb = wk.tile([128, B], F32R, tag="hd_ctb")
                        nc.vector.tensor_copy(hd_ctb[:], tp[:])
                        nc.sync.dma_start(bufs["d_in"].ap(),
                                          hd_ctb[:].bitcast(F32))
                        nc.gpsimd.collective_compute(
                            "AllGather", mybir.AluOpType.bypass,
                            replica_groups=RG,
                            ins=[bufs["d_in"].ap().opt()],
                            outs=[bufs["d_out"].ap().opt()])
                        for c in range(2):
                            src = bufs["d_out"].ap()[c * 512:(c + 1) * 512, :]
                            nc.sync.dma_start(
                                hdT[:, c * 4 * B:(c + 1) * 4 * B]
                                .rearrange("p (t n) -> p t n", t=4),
                                src.rearrange("(t p) n -> p t n", t=4)
                                .bitcast(F32R))

                    # -------------- epilogue --------------
                    cd_ps = ps.tile([B, LS], F32, tag="mm")
                    for kt in range(KH):
                        nc.tensor.matmul(
                            cd_ps[:], hdT[:, kt * B:(kt + 1) * B],
                            wwr_sb[:, kt * LS:(kt + 1) * LS],
                            start=(kt == 0), stop=(kt == KH - 1),
                            skip_group_check=True)
                    nc.vector.tensor_add(c_t[:], c_t[:], cd_ps[:])
                    nc.vector.tensor_add(c_t[:], c_t[:], bw_sb[:])
                    if debug and rep == 0:
                        nc.sync.dma_start(dbg["dbg_ct"], c_t[:])
                        nc.sync.dma_start(
                            dbg["dbg_heT"].rearrange("(t p) n -> p t n", t=KH),
                            heT[:].rearrange("p (t n) -> p t n", t=KH)
                            .bitcast(F32))
                        nc.sync.dma_start(
                            dbg["dbg_hdT"].rearrange("(t p) n -> p t n", t=KH),
                            hdT[:].rearrange("p (t n) -> p t n", t=KH)
                            .bitcast(F32))
                    o_sb = wk.tile([B, LS], F32, tag="s_b")
                    nc.scalar.activation(o_sb[:], c_t[:], AFT.Sigmoid)
                    nc.sync.dma_start(out, o_sb[:])

    nc.compile()
    return nc


def _prep_inputs(inputs):
    """Build the 8 per-core input maps from the full problem inputs."""
    f = np.float32
    bf = ml_dtypes.bfloat16
    x = np.asarray(inputs["x"], f)
    eps = np.ascontiguousarray(np.asarray(inputs["eps"], f))
    Wie = np.asarray(inputs["W_ih_enc"], f)
    Whe = np.asarray(inputs["W_hh_enc"], f)
    Wms = np.concatenate([np.asarray(inputs["W_mu"], f),
                          np.asarray(inputs["W_sigma"], f)], 0)
    bms = np.concatenate([np.asarray(inputs["b_mu"], f),
                          np.asarray(inputs["b_sigma"], f)], 0)
    Wid = np.asarray(inputs["W_ih_dec"], f)
    Whd = np.asarray(inputs["W_hh_dec"], f)
    Wwr = np.asarray(inputs["W_write"], f)
    be = np.asarray(inputs["b_ih_enc"], f) + np.asarray(inputs["b_hh_enc"], f)
    bd = np.asarray(inputs["b_ih_dec"], f) + np.asarray(inputs["b_hh_dec"], f)
    bw = np.asarray(inputs["b_write"], f)
    c0 = np.asarray(inputs["c_0"], f)
    h0e = np.asarray(inputs["h_0_enc"], f)
    h0d = np.asarray(inputs["h_0_dec"], f)

    xT = np.ascontiguousarray(x.T)
    h0eT = np.ascontiguousarray(np.repeat(h0e[:, None], B, 1))
    h0dT = np.ascontiguousarray(np.repeat(h0d[:, None], B, 1))
    wmst = np.ascontiguousarray(Wms.T)
    bms_b = np.ascontiguousarray(np.broadcast_to(bms, (B, GS)))

    in_maps = []
    for k in range(NCORES):
        Gk = np.concatenate(
            [np.arange(HS * k, HS * (k + 1)) + j * H for j in range(4)])
        Lk = slice(LS * k, LS * (k + 1))
        W1g = Wie[Gk, :L]
        W2g = Wie[Gk, L:2 * L]
        in_maps.append({
            "xT": xT,
            "w12t": np.ascontiguousarray((W1g + W2g).T),
            "w2t": np.ascontiguousarray(-W2g.T).astype(bf),
            "w3t": np.ascontiguousarray(Wie[Gk, 2 * L:].T),
            "whht": np.ascontiguousarray(Whe[Gk].T),
            "wmst": wmst,
            "widt": np.ascontiguousarray(Wid[Gk].T),
            "whdt": np.ascontiguousarray(Whd[Gk].T),
            "wwrt": np.ascontiguousarray(Wwr[Lk].T),
            "bias_e": np.ascontiguousarray(np.broadcast_to(be[Gk], (B, GS))),
            "bias_ms": bms_b,
            "bias_d": np.ascontiguousarray(np.broadcast_to(bd[Gk], (B, GS))),
            "bias_w": np.ascontiguousarray(np.broadcast_to(bw[Lk], (B, LS))),
            "c0b": np.ascontiguousarray(np.broadcast_to(c0[0, Lk], (B, LS))),
            "h0eT": h0eT,
            "h0dT": h0dT,
            "eps": eps,
        })
    return in_maps


_NC_CACHE = {}


def _get_nc(repeats=1, debug=False):
    key = (repeats, debug)
    if key not in _NC_CACHE:
        _NC_CACHE[key] = build(repeats=repeats, debug=debug)
    return _NC_CACHE[key]


def run(inputs, repeats=1, debug=False):
    nc = _get_nc(repeats=repeats, debug=debug)
    in_maps = _prep_inputs(inputs)
    res = run_bass_kernel_spmd(nc, in_maps, core_ids=list(range(NCORES)))
    full = np.concatenate([res.results[k]["out"] for k in range(NCORES)], 1)
    return full.astype(np.float32), res


def kernel(**inputs) -> np.ndarray:
    out, _ = run(inputs, repeats=1, debug=False)
    return out


# revision 11
# speedup vs baseline: 53.0963x; 53.0963x over previous
"""DRAW-style recurrent VAE kernel for 8 Trainium2 NeuronCores.

Self-contained: takes FULL inputs (as produced by setup_inputs()), returns the
FULL (B, L) output of sigmoid(c_T).

Sharding (model-parallel, NOT data-parallel -- the 200MB of weights do not fit
in one core's SBUF replicated, and TensorE matmul cost is independent of the
batch/M dim so batch sharding saves nothing):
  - core k owns gate rows Gk = [Hk, H+Hk, 2H+Hk, 3H+Hk] (512 of 4096 gates,
    i.e. h-slice Hk = [128k, 128k+128) of both LSTMs) and canvas columns
    Lk = [512k, 512k+512).
  - per step: 3 AllGathers (S^T slices in bf16, h_enc^T slice, h_dec^T slice).
  - x @ (W1+W2).T is constant across timesteps -> computed once in a preamble
    (r_t = [x, x - sigmoid(c_t), h_dec] so W_ih@r = (W1+W2)@x - W2@S + W3@h).
  - matmuls run in float32r (full PE rate); the big -S@W2 term runs in bf16
    (W2 resident bf16 halves SBUF + AG traffic); exp() computed as
    sigmoid(x)/sigmoid(-x) to stay in the sigmoid+tanh ACT table set.
"""

import numpy as np
import ml_dtypes

import concourse.bass as bass  # noqa: F401
import concourse.mybir as mybir
import concourse.tile as tile
from concourse import bacc
from concourse.bass_utils import run_bass_kernel_spmd
from concourse.masks import make_identity

F32 = mybir.dt.float32
F32R = mybir.dt.float32r
BF16 = mybir.dt.bfloat16
AFT = mybir.ActivationFunctionType

B, L, H, Z, T = 128, 4096, 1024, 256, 16
NCORES = 8
LS = L // NCORES          # 512 canvas cols per core
HS = H // NCORES          # 128 h rows per core
GS = 4 * HS               # 512 gate rows per core
KL = L // 128             # 32 K-tiles over L
KH = H // 128             # 8  K-tiles over H
KZ = Z // 128             # 2  K-tiles over Z


def _load_T(nc, sb_tile, dram_ap, ktiles, n, nchunks):
    """DRAM (ktiles*128, n) row-major -> SBUF tile [128, ktiles*n] where
    free-dim slice [kt*n:(kt+1)*n] holds K-tile kt. Split into nchunks DMAs."""
    per = ktiles // nchunks
    assert per * nchunks == ktiles
    for c in range(nchunks):
        src = dram_ap[c * per * 128:(c + 1) * per * 128, :]
        src = src.rearrange("(t p) n -> p t n", t=per)
        dst = sb_tile[:, c * per * n:(c + 1) * per * n]
        dst = dst.rearrange("p (t n) -> p t n", t=per)
        nc.sync.dma_start(dst, src)


def build(repeats=1, debug=False, cc_mode="real", s_split=2,
          zb_ms=False, zb_d=False, zb_w=False):
    nc = bacc.Bacc("TRN2", target_bir_lowering=False, debug=False,
                   num_devices=NCORES)

    def inp(name, shape, dt=F32R):
        return nc.dram_tensor(name, list(shape), dt, kind="ExternalInput").ap()

    xT = inp("xT", (L, B))                  # x.T (replicated)
    w12t = inp("w12t", (L, GS))             # (W1+W2)[Gk].T (streamed, preamble)
    w2t = inp("w2t", (L, GS), BF16)         # -W2[Gk].T  (resident, negated!)
    w3t = inp("w3t", (H, GS))
    whht = inp("whht", (H, GS))
    wmst = inp("wmst", (H, GS))             # [W_mu; W_sigma].T (replicated)
    widt = inp("widt", (Z, GS))
    whdt = inp("whdt", (H, GS))
    wwrt = inp("wwrt", (H, LS))             # W_write[Lk].T
    bias_e = inp("bias_e", (B, GS), F32)    # (b_ih_enc+b_hh_enc)[Gk] bcast
    bias_ms = inp("bias_ms", (B, GS), F32)
    bias_d = inp("bias_d", (B, GS), F32)
    bias_w = inp("bias_w", (B, LS), F32)
    c0b = inp("c0b", (B, LS), F32)          # c_0[Lk] broadcast over batch
    h0eT = inp("h0eT", (H, B))              # h_0_enc.T bcast
    h0dT = inp("h0dT", (H, B))
    eps_all = inp("eps", (T, B, Z), F32)
    s0T = inp("s0T", (L, B), BF16)          # sigmoid(c_0).T bcast (step 0)

    out = nc.dram_tensor("out", [B, LS], F32, kind="ExternalOutput").ap()
    dbg = {}
    if debug:
        for nm, shape in [("dbg_heT", (H, B)), ("dbg_hdT", (H, B)),
                          ("dbg_ct", (B, LS)), ("dbg_ms", (B, GS)),
                          ("dbg_ge", (B, GS))]:
            dbg[nm] = nc.dram_tensor(nm, list(shape), F32,
                                     kind="ExternalOutput").ap()

    # collective bounce buffers (double-buffered across steps)
    ag = []
    for i in range(2):
        ag.append({
            "s_in": [nc.dram_tensor(f"ag_s_in{i}_{h}", [LS // s_split, B], BF16)
                     for h in range(s_split)],
            "s_out": [nc.dram_tensor(f"ag_s_out{i}_{h}", [L // s_split, B],
                                     BF16, addr_space="Shared")
                      for h in range(s_split)],
            "e_in": nc.dram_tensor(f"ag_e_in{i}", [HS, B], F32),
            "e_out": nc.dram_tensor(f"ag_e_out{i}", [H, B], F32,
                                    addr_space="Shared"),
            "d_in": nc.dram_tensor(f"ag_d_in{i}", [HS, B], F32),
            "d_out": nc.dram_tensor(f"ag_d_out{i}", [H, B], F32,
                                    addr_space="Shared"),
        })

    RG = [list(range(NCORES))]

    def allgather(in_t, out_t, nrows):
        if cc_mode == "real":
            nc.gpsimd.collective_compute(
                "AllGather", mybir.AluOpType.bypass,
                replica_groups=RG,
                ins=[in_t.ap().opt()],
                outs=[out_t.ap().opt()])
        else:  # selfcopy: timing bisect only (numerics wrong)
            for r in range(NCORES):
                nc.sync.dma_start(
                    out_t.ap()[r * nrows:(r + 1) * nrows, :], in_t.ap())

    with tile.TileContext(nc) as tc:
        with tc.tile_pool(name="weights", bufs=1) as wp, \
             tc.tile_pool(name="state", bufs=1) as st, \
             tc.tile_pool(name="mmps", bufs=4, space="PSUM") as ps, \
             tc.tile_pool(name="trps", bufs=3, space="PSUM") as tr, \
             tc.tile_pool(name="epspool", bufs=2) as epp:

            # ---------------- resident weights ----------------
            w2_sb = wp.tile([128, KL * GS], BF16)    # 32KB/part
            _load_T(nc, w2_sb, w2t, KL, GS, 8)
            w3_sb = wp.tile([128, KH * GS], F32R)
            _load_T(nc, w3_sb, w3t, KH, GS, 2)
            whh_sb = wp.tile([128, KH * GS], F32R)
            _load_T(nc, whh_sb, whht, KH, GS, 2)
            wms_sb = wp.tile([128, KH * GS], F32R)
            _load_T(nc, wms_sb, wmst, KH, GS, 2)
            wid_sb = wp.tile([128, KZ * GS], F32R)
            _load_T(nc, wid_sb, widt, KZ, GS, 1)
            whd_sb = wp.tile([128, KH * GS], F32R)
            _load_T(nc, whd_sb, whdt, KH, GS, 2)
            wwr_sb = wp.tile([128, KH * LS], F32R)
            _load_T(nc, wwr_sb, wwrt, KH, LS, 2)

            be_sb = wp.tile([B, GS], F32)
            nc.sync.dma_start(be_sb[:], bias_e)
            bms_sb = None
            if not zb_ms:
                bms_sb = wp.tile([B, GS], F32)
                nc.sync.dma_start(bms_sb[:], bias_ms)
            bd_sb = None
            if not zb_d:
                bd_sb = wp.tile([B, GS], F32)
                nc.sync.dma_start(bd_sb[:], bias_d)
            bw_sb = None
            if not zb_w:
                bw_sb = wp.tile([B, LS], F32)
                nc.sync.dma_start(bw_sb[:], bias_w)

            ident = wp.tile([128, 128], F32)
            make_identity(nc, ident[:])

            # persistent state
            c_t = st.tile([B, LS], F32)
            c_enc = st.tile([B, HS], F32)
            c_dec = st.tile([B, HS], F32)
            heT = st.tile([128, KH * B], F32R)   # h_enc.T full (8 slots)
            hdT = st.tile([128, KH * B], F32R)
            sT = st.tile([128, KL * B], BF16)    # S.T full (32 slots)
            const_sb = st.tile([B, GS], F32)

            for rep in range(repeats):
                # -------------- init state --------------
                nc.sync.dma_start(c_t[:], c0b)
                nc.gpsimd.memset(c_enc[:], 0.0)
                nc.gpsimd.memset(c_dec[:], 0.0)
                _load_T(nc, heT, h0eT, KH, B, 2)
                _load_T(nc, hdT, h0dT, KH, B, 2)

                # ---- preamble: const = x@(W1+W2)[Gk].T + bias_e ----
                with tc.tile_pool(name="pre", bufs=3) as pre:
                    cst_ps = ps.tile([B, GS], F32, tag="mm")
                    for c in range(8):
                        xT_sb = pre.tile([128, 4 * B], F32R, tag="xT")
                        src = xT[c * 512:(c + 1) * 512, :]
                        nc.sync.dma_start(
                            xT_sb[:].rearrange("p (t n) -> p t n", t=4),
                            src.rearrange("(t p) n -> p t n", t=4))
                        w12_sb = pre.tile([128, 4 * GS], F32R, tag="w12")
                        src = w12t[c * 512:(c + 1) * 512, :]
                        nc.sync.dma_start(
                            w12_sb[:].rearrange("p (t n) -> p t n", t=4),
                            src.rearrange("(t p) n -> p t n", t=4))
                        for j in range(4):
                            kt = c * 4 + j
                            nc.tensor.matmul(
                                cst_ps[:],
                                xT_sb[:, j * B:(j + 1) * B],
                                w12_sb[:, j * GS:(j + 1) * GS],
                                start=(kt == 0), stop=(kt == KL - 1))
                    nc.vector.tensor_add(const_sb[:], cst_ps[:], be_sb[:])

                # -------------- time loop --------------
                with tc.tile_pool(name="work", bufs=2) as wk:
                    for t in range(T):
                        bufs = ag[t % 2]
                        eps_sb = epp.tile([B, Z], F32, tag="eps")
                        nc.sync.dma_start(eps_sb[:], eps_all[t])
                        # (1) enc-gate PSUM opens with Whh (h_encT[t-1] is
                        # available early) -> PE busy while AG(h_dec) lands.
                        ge_ps = ps.tile([B, GS], F32, tag="mm")
                        for kt in range(KH):
                            nc.tensor.matmul(
                                ge_ps[:], heT[:, kt * B:(kt + 1) * B],
                                whh_sb[:, kt * GS:(kt + 1) * GS],
                                start=(kt == 0), stop=False,
                                skip_group_check=True)

                        # (2) canvas update: c_t += h_dec@W_write.T + b_w
                        if t > 0:
                            cd_ps = ps.tile([B, LS], F32, tag="mm")
                            for kt in range(KH):
                                nc.tensor.matmul(
                                    cd_ps[:], hdT[:, kt * B:(kt + 1) * B],
                                    wwr_sb[:, kt * LS:(kt + 1) * LS],
                                    start=(kt == 0), stop=(kt == KH - 1),
                                    skip_group_check=True)
                            nc.vector.tensor_add(c_t[:], c_t[:], cd_ps[:])
                            if not zb_w:
                                nc.vector.tensor_add(c_t[:], c_t[:], bw_sb[:])

                        # (3) S = sigmoid(c_t); transpose; AG contribution
                        # t=0: S is known on the host (sigmoid of c_0) --
                        # preload the full S.T and skip the gather.
                        if t > 0:
                            s_b = wk.tile([B, LS], F32, tag="s_b")
                            nc.scalar.activation(s_b[:], c_t[:], AFT.Sigmoid)
                            s_ctb = wk.tile([128, 4 * B], BF16, tag="s_ctb")
                            for j in range(4):
                                tp = tr.tile([128, 128], F32, tag="tr")
                                nc.tensor.transpose(
                                    tp[:], s_b[:, j * 128:(j + 1) * 128],
                                    ident[:])
                                nc.vector.tensor_copy(
                                    s_ctb[:, j * B:(j + 1) * B], tp[:])
                            tph = 4 // s_split   # local tiles per split
                            for h in range(s_split):
                                nc.sync.dma_start(
                                    bufs["s_in"][h].ap()
                                    .rearrange("(t p) n -> p t n", t=tph),
                                    s_ctb[:, h * tph * B:(h + 1) * tph * B]
                                    .rearrange("p (t n) -> p t n", t=tph))
                                allgather(bufs["s_in"][h], bufs["s_out"][h],
                                          LS // s_split)

                        # (4) W3 term (h_decT): fills AG(S) latency
                        for kt in range(KH):
                            nc.tensor.matmul(
                                ge_ps[:], hdT[:, kt * B:(kt + 1) * B],
                                w3_sb[:, kt * GS:(kt + 1) * GS],
                                start=False, stop=False, skip_group_check=True)

                        # (5) load S.T full; -S@W2 accumulation (32 MMs)
                        if t == 0:
                            _load_T(nc, sT, s0T, KL, B, 8)
                        else:
                            for h in range(s_split):
                                for c in range(8):
                                    rows = (LS // s_split)
                                    src = bufs["s_out"][h].ap()[
                                        c * rows:(c + 1) * rows, :]
                                    slot = c * 4 + h * tph
                                    nc.sync.dma_start(
                                        sT[:, slot * B:(slot + tph) * B]
                                        .rearrange("p (t n) -> p t n", t=tph),
                                        src.rearrange("(t p) n -> p t n",
                                                      t=tph))
                        for kt in range(KL):
                            nc.tensor.matmul(
                                ge_ps[:], sT[:, kt * B:(kt + 1) * B],
                                w2_sb[:, kt * GS:(kt + 1) * GS],
                                start=False, stop=(kt == KL - 1),
                                skip_group_check=True)

                        # (6) encoder LSTM pointwise
                        ge = wk.tile([B, GS], F32, tag="ge")
                        nc.vector.tensor_add(ge[:], ge_ps[:], const_sb[:])
                        if debug and t == 0 and rep == 0:
                            nc.sync.dma_start(dbg["dbg_ge"], ge[:])
                        sif = wk.tile([B, 384], F32, tag="sif")
                        nc.scalar.activation(sif[:], ge[:, 0:384], AFT.Sigmoid)
                        tg = wk.tile([B, HS], F32, tag="tg")
                        nc.scalar.activation(tg[:], ge[:, 384:512], AFT.Tanh)
                        t2 = wk.tile([B, HS], F32, tag="t2")
                        nc.vector.tensor_mul(t2[:], sif[:, 0:128], tg[:])
                        nc.vector.tensor_mul(c_enc[:], c_enc[:],
                                             sif[:, 128:256])
                        nc.vector.tensor_add(c_enc[:], c_enc[:], t2[:])
                        tc_ = wk.tile([B, HS], F32, tag="tc_")
                        nc.scalar.activation(tc_[:], c_enc[:], AFT.Tanh)
                        h_e = wk.tile([B, HS], F32, tag="h_e")
                        nc.vector.tensor_mul(h_e[:], sif[:, 256:384], tc_[:])

                        # (7) transpose h_enc; AG
                        tp = tr.tile([128, 128], F32, tag="tr")
                        nc.tensor.transpose(tp[:], h_e[:], ident[:])
                        he_ctb = wk.tile([128, B], F32R, tag="he_ctb")
                        nc.vector.tensor_copy(he_ctb[:], tp[:])
                        nc.sync.dma_start(bufs["e_in"].ap(),
                                          he_ctb[:].bitcast(F32))
                        allgather(bufs["e_in"], bufs["e_out"], HS)

                        # (8) dec-gate PSUM opens with Whd (h_decT[t-1]):
                        # fills AG(h_enc) latency
                        gd_ps = ps.tile([B, GS], F32, tag="mm")
                        for kt in range(KH):
                            nc.tensor.matmul(
                                gd_ps[:], hdT[:, kt * B:(kt + 1) * B],
                                whd_sb[:, kt * GS:(kt + 1) * GS],
                                start=(kt == 0), stop=False,
                                skip_group_check=True)

                        # (9) reload h_encT full; musig
                        for c in range(2):
                            src = bufs["e_out"].ap()[c * 512:(c + 1) * 512, :]
                            nc.sync.dma_start(
                                heT[:, c * 4 * B:(c + 1) * 4 * B]
                                .rearrange("p (t n) -> p t n", t=4),
                                src.rearrange("(t p) n -> p t n", t=4)
                                .bitcast(F32R))
                        ms_ps = ps.tile([B, GS], F32, tag="mm")
                        for kt in range(KH):
                            nc.tensor.matmul(
                                ms_ps[:], heT[:, kt * B:(kt + 1) * B],
                                wms_sb[:, kt * GS:(kt + 1) * GS],
                                start=(kt == 0), stop=(kt == KH - 1),
                                skip_group_check=True)
                        if zb_ms and not debug:
                            ms = ms_ps
                        else:
                            ms = wk.tile([B, GS], F32, tag="ms")
                            if zb_ms:
                                nc.vector.tensor_copy(ms[:], ms_ps[:])
                            else:
                                nc.vector.tensor_add(ms[:], ms_ps[:],
                                                     bms_sb[:])
                            if debug and t == T - 1 and rep == 0:
                                nc.sync.dma_start(dbg["dbg_ms"], ms[:])

                        # (10) z = eps*exp(logsig) + mu; exp = sig(x)/sig(-x)
                        spz = wk.tile([B, Z], F32, tag="spz")
                        nc.scalar.activation(spz[:], ms[:, 256:512],
                                             AFT.Sigmoid)
                        snz = wk.tile([B, Z], F32, tag="snz")
                        nc.scalar.activation(snz[:], ms[:, 256:512],
                                             AFT.Sigmoid, scale=-1.0)
                        rnz = wk.tile([B, Z], F32, tag="rnz")
                        nc.vector.reciprocal(rnz[:], snz[:])
                        nc.vector.tensor_mul(rnz[:], spz[:], rnz[:])
                        nc.vector.tensor_mul(rnz[:], eps_sb[:], rnz[:])
                        z = wk.tile([B, Z], F32, tag="z")
                        nc.vector.tensor_add(z[:], rnz[:], ms[:, 0:256])

                        # (11) z.T (2 tiles); finish decoder gates with Wid
                        zT = wk.tile([128, KZ * B], F32R, tag="zT")
                        for j in range(KZ):
                            tp = tr.tile([128, 128], F32, tag="tr")
                            nc.tensor.transpose(
                                tp[:], z[:, j * 128:(j + 1) * 128], ident[:])
                            nc.vector.tensor_copy(
                                zT[:, j * B:(j + 1) * B], tp[:])
                        for kt in range(KZ):
                            nc.tensor.matmul(
                                gd_ps[:], zT[:, kt * B:(kt + 1) * B],
                                wid_sb[:, kt * GS:(kt + 1) * GS],
                                start=False, stop=(kt == KZ - 1),
                                skip_group_check=True)

                        # (12) decoder LSTM pointwise
                        if zb_d:
                            gd = gd_ps
                        else:
                            gd = wk.tile([B, GS], F32, tag="gd")
                            nc.vector.tensor_add(gd[:], gd_ps[:], bd_sb[:])
                        sifd = wk.tile([B, 384], F32, tag="sifd")
                        nc.scalar.activation(sifd[:], gd[:, 0:384],
                                             AFT.Sigmoid)
                        tgd = wk.tile([B, HS], F32, tag="tgd")
                        nc.scalar.activation(tgd[:], gd[:, 384:512], AFT.Tanh)
                        t2d = wk.tile([B, HS], F32, tag="t2d")
                        nc.vector.tensor_mul(t2d[:], sifd[:, 0:128], tgd[:])
                        nc.vector.tensor_mul(c_dec[:], c_dec[:],
                                             sifd[:, 128:256])
                        nc.vector.tensor_add(c_dec[:], c_dec[:], t2d[:])
                        tcd = wk.tile([B, HS], F32, tag="tcd")
                        nc.scalar.activation(tcd[:], c_dec[:], AFT.Tanh)
                        h_d = wk.tile([B, HS], F32, tag="h_d")
                        nc.vector.tensor_mul(h_d[:], sifd[:, 256:384], tcd[:])

                        # (13) transpose h_dec; AG; reload h_decT full
                        tp = tr.tile([128, 128], F32, tag="tr")
                        nc.tensor.transpose(tp[:], h_d[:], ident[:])
                        hd_ctb = wk.tile([128, B], F32R, tag="hd_ctb")
                        nc.vector.tensor_copy(hd_ctb[:], tp[:])
                        nc.sync.dma_start(bufs["d_in"].ap(),
                                          hd_ctb[:].bitcast(F32))
                        allgather(bufs["d_in"], bufs["d_out"], HS)
                        for c in range(2):
                            src = bufs["d_out"].ap()[c * 512:(c + 1) * 512, :]
                            nc.sync.dma_start(
                                hdT[:, c * 4 * B:(c + 1) * 4 * B]
                                .rearrange("p (t n) -> p t n", t=4),
                                src.rearrange("(t p) n -> p t n", t=4)
                                .bitcast(F32R))

                    # -------------- epilogue --------------
                    cd_ps = ps.tile([B, LS], F32, tag="mm")
                    for kt in range(KH):
                        nc.tensor.matmul(
                            cd_ps[:], hdT[:, kt * B:(kt + 1) * B],
                            wwr_sb[:, kt * LS:(kt + 1) * LS],
                            start=(kt == 0), stop=(kt == KH - 1),
                            skip_group_check=True)
                    nc.vector.tensor_add(c_t[:], c_t[:], cd_ps[:])
                    if not zb_w:
                        nc.vector.tensor_add(c_t[:], c_t[:], bw_sb[:])
                    if debug and rep == 0:
                        nc.sync.dma_start(dbg["dbg_ct"], c_t[:])
                        nc.sync.dma_start(
                            dbg["dbg_heT"].rearrange("(t p) n -> p t n", t=KH),
                            heT[:].rearrange("p (t n) -> p t n", t=KH)
                            .bitcast(F32))
                        nc.sync.dma_start(
                            dbg["dbg_hdT"].rearrange("(t p) n -> p t n", t=KH),
                            hdT[:].rearrange("p (t n) -> p t n", t=KH)
                            .bitcast(F32))
                    o_sb = wk.tile([B, LS], F32, tag="s_b")
                    nc.scalar.activation(o_sb[:], c_t[:], AFT.Sigmoid)
                    nc.sync.dma_start(out, o_sb[:])

    nc.compile()
    return nc


def _prep_inputs(inputs):
    """Build the 8 per-core input maps from the full problem inputs."""
    f = np.float32
    bf = ml_dtypes.bfloat16
    x = np.asarray(inputs["x"], f)
    eps = np.ascontiguousarray(np.asarray(inputs["eps"], f))
    Wie = np.asarray(inputs["W_ih_enc"], f)
    Whe = np.asarray(inputs["W_hh_enc"], f)
    Wms = np.concatenate([np.asarray(inputs["W_mu"], f),
                          np.asarray(inputs["W_sigma"], f)], 0)
    bms = np.concatenate([np.asarray(inputs["b_mu"], f),
                          np.asarray(inputs["b_sigma"], f)], 0)
    Wid = np.asarray(inputs["W_ih_dec"], f)
    Whd = np.asarray(inputs["W_hh_dec"], f)
    Wwr = np.asarray(inputs["W_write"], f)
    be = np.asarray(inputs["b_ih_enc"], f) + np.asarray(inputs["b_hh_enc"], f)
    bd = np.asarray(inputs["b_ih_dec"], f) + np.asarray(inputs["b_hh_dec"], f)
    bw = np.asarray(inputs["b_write"], f)
    c0 = np.asarray(inputs["c_0"], f)
    h0e = np.asarray(inputs["h_0_enc"], f)
    h0d = np.asarray(inputs["h_0_dec"], f)

    xT = np.ascontiguousarray(x.T)
    h0eT = np.ascontiguousarray(np.repeat(h0e[:, None], B, 1))
    h0dT = np.ascontiguousarray(np.repeat(h0d[:, None], B, 1))
    wmst = np.ascontiguousarray(Wms.T)
    bms_b = np.ascontiguousarray(np.broadcast_to(bms, (B, GS)))

    s0 = 1.0 / (1.0 + np.exp(-c0[0].astype(np.float64)))
    s0T = np.ascontiguousarray(
        np.repeat(s0[:, None].astype(f), B, 1)).astype(bf)

    in_maps = []
    for k in range(NCORES):
        # gate row order [i, f, o, g] so sigmoid covers one contiguous block
        Gk = np.concatenate(
            [np.arange(HS * k, HS * (k + 1)) + j * H for j in (0, 1, 3, 2)])
        Lk = slice(LS * k, LS * (k + 1))
        W1g = Wie[Gk, :L]
        W2g = Wie[Gk, L:2 * L]
        in_maps.append({
            "xT": xT,
            "w12t": np.ascontiguousarray((W1g + W2g).T),
            "w2t": np.ascontiguousarray(-W2g.T).astype(bf),
            "w3t": np.ascontiguousarray(Wie[Gk, 2 * L:].T),
            "whht": np.ascontiguousarray(Whe[Gk].T),
            "wmst": wmst,
            "widt": np.ascontiguousarray(Wid[Gk].T),
            "whdt": np.ascontiguousarray(Whd[Gk].T),
            "wwrt": np.ascontiguousarray(Wwr[Lk].T),
            "bias_e": np.ascontiguousarray(np.broadcast_to(be[Gk], (B, GS))),
            "bias_ms": bms_b,
            "bias_d": np.ascontiguousarray(np.broadcast_to(bd[Gk], (B, GS))),
            "bias_w": np.ascontiguousarray(np.broadcast_to(bw[Lk], (B, LS))),
            "c0b": np.ascontiguousarray(np.broadcast_to(c0[0, Lk], (B, LS))),
            "h0eT": h0eT,
            "h0dT": h0dT,
            "eps": eps,
            "s0T": s0T,
        })
    zb = {"zb_ms": not np.any(bms), "zb_d": not np.any(bd),
          "zb_w": not np.any(bw)}
    return in_maps, zb


_NC_CACHE = {}


def _get_nc(repeats=1, debug=False, cc_mode="real", s_split=2, **zb):
    key = (repeats, debug, cc_mode, s_split, tuple(sorted(zb.items())))
    if key not in _NC_CACHE:
        _NC_CACHE[key] = build(repeats=repeats, debug=debug, cc_mode=cc_mode,
                               s_split=s_split, **zb)
    return _NC_CACHE[key]


def run(inputs, repeats=1, debug=False, cc_mode="real", s_split=2):
    in_maps, zb = _prep_inputs(inputs)
    nc = _get_nc(repeats=repeats, debug=debug, cc_mode=cc_mode,
                 s_split=s_split, **zb)
    res = run_bass_kernel_spmd(nc, in_maps, core_ids=list(range(NCORES)))
    full = np.concatenate([res.results[k]["out"] for k in range(NCORES)], 1)
    return full.astype(np.float32), res


def kernel(**inputs) -> np.ndarray:
    out, _ = run(inputs, repeats=1, debug=False)
    return out


# revision 13
# speedup vs baseline: 173.4043x; 3.2658x over previous
"""DRAW-style recurrent VAE kernel for 8 Trainium2 NeuronCores.

Self-contained: takes FULL inputs (as produced by setup_inputs()), returns the
FULL (B, L) output of sigmoid(c_T).

Sharding (model-parallel, NOT data-parallel -- the 200MB of weights do not fit
in one core's SBUF replicated, and TensorE matmul cost is independent of the
batch/M dim so batch sharding saves nothing):
  - core k owns gate rows Gk (512 of 4096 gates in [i,f,o,g] order, i.e.
    h-slice Hk = [128k, 128k+128) of both LSTMs) and canvas columns
    Lk = [512k, 512k+512).
  - per step: 6 AllGathers (S^T in four bf16 quarters, h_enc^T, h_dec^T).
    The S gather is SPLIT because large-output AllGathers hit slow collective
    paths in this runtime (1MB out: ~22ms; 512KB bf16 out: ~170us; 256KB out:
    ~6-8us == near the floor). Four 256KB-out quarters win decisively.
  - x @ (W1+W2).T is constant across timesteps -> computed once in a preamble
    (r_t = [x, x - sigmoid(c_t), h_dec] so W_ih@r = (W1+W2)@x - W2@S + W3@h);
    sigmoid(c_0) is known on the host, so step 0 skips the S gather.
  - matmuls run in float32r (full PE rate at N>=256); the big -S@W2 term runs
    in bf16 (W2 negated host-side; bf16 halves SBUF + AG traffic); exp()
    computed as sigmoid(x)/sigmoid(-x) to stay in the sigmoid+tanh ACT set.
  - biases are all zero in setup_inputs(); build() specializes the zero-bias
    adds away but stays correct for nonzero biases via the zb_* flags.
"""

import numpy as np
import ml_dtypes

import concourse.bass as bass  # noqa: F401
import concourse.mybir as mybir
import concourse.tile as tile
from concourse import bacc
from concourse.bass_utils import run_bass_kernel_spmd
from concourse.masks import make_identity

F32 = mybir.dt.float32
F32R = mybir.dt.float32r
BF16 = mybir.dt.bfloat16
AFT = mybir.ActivationFunctionType

B, L, H, Z, T = 128, 4096, 1024, 256, 16
NCORES = 8
LS = L // NCORES          # 512 canvas cols per core
HS = H // NCORES          # 128 h rows per core
GS = 4 * HS               # 512 gate rows per core
KL = L // 128             # 32 K-tiles over L
KH = H // 128             # 8  K-tiles over H
KZ = Z // 128             # 2  K-tiles over Z


def _load_T(nc, sb_tile, dram_ap, ktiles, n, nchunks):
    """DRAM (ktiles*128, n) row-major -> SBUF tile [128, ktiles*n] where
    free-dim slice [kt*n:(kt+1)*n] holds K-tile kt. Split into nchunks DMAs."""
    per = ktiles // nchunks
    assert per * nchunks == ktiles
    for c in range(nchunks):
        src = dram_ap[c * per * 128:(c + 1) * per * 128, :]
        src = src.rearrange("(t p) n -> p t n", t=per)
        dst = sb_tile[:, c * per * n:(c + 1) * per * n]
        dst = dst.rearrange("p (t n) -> p t n", t=per)
        nc.sync.dma_start(dst, src)


def build(repeats=1, debug=False, cc_mode="real", s_split=4,
          zb_ms=False, zb_d=False, zb_w=False):
    nc = bacc.Bacc("TRN2", target_bir_lowering=False, debug=False,
                   num_devices=NCORES)

    def inp(name, shape, dt=F32R):
        return nc.dram_tensor(name, list(shape), dt, kind="ExternalInput").ap()

    xT = inp("xT", (L, B))                  # x.T (replicated)
    w12t = inp("w12t", (L, GS))             # (W1+W2)[Gk].T (streamed, preamble)
    w2t = inp("w2t", (L, GS), BF16)         # -W2[Gk].T  (resident, negated!)
    w3t = inp("w3t", (H, GS))
    whht = inp("whht", (H, GS))
    wmst = inp("wmst", (H, GS))             # [W_mu; W_sigma].T (replicated)
    widt = inp("widt", (Z, GS))
    whdt = inp("whdt", (H, GS))
    wwrt = inp("wwrt", (H, LS))             # W_write[Lk].T
    bias_e = inp("bias_e", (B, GS), F32)    # (b_ih_enc+b_hh_enc)[Gk] bcast
    bias_ms = inp("bias_ms", (B, GS), F32)
    bias_d = inp("bias_d", (B, GS), F32)
    bias_w = inp("bias_w", (B, LS), F32)
    c0b = inp("c0b", (B, LS), F32)          # c_0[Lk] broadcast over batch
    h0eT = inp("h0eT", (H, B))              # h_0_enc.T bcast
    h0dT = inp("h0dT", (H, B))
    eps_all = inp("eps", (T, B, Z), F32)
    s0T = inp("s0T", (L, B), BF16)          # sigmoid(c_0).T bcast (step 0)

    out = nc.dram_tensor("out", [B, LS], F32, kind="ExternalOutput").ap()
    dbg = {}
    if debug:
        for nm, shape in [("dbg_heT", (H, B)), ("dbg_hdT", (H, B)),
                          ("dbg_ct", (B, LS)), ("dbg_ms", (B, GS)),
                          ("dbg_ge", (B, GS))]:
            dbg[nm] = nc.dram_tensor(nm, list(shape), F32,
                                     kind="ExternalOutput").ap()

    # collective bounce buffers (double-buffered across steps)
    ag = []
    for i in range(2):
        ag.append({
            "s_in": [nc.dram_tensor(f"ag_s_in{i}_{h}", [LS // s_split, B], BF16)
                     for h in range(s_split)],
            "s_out": [nc.dram_tensor(f"ag_s_out{i}_{h}", [L // s_split, B],
                                     BF16, addr_space="Shared")
                      for h in range(s_split)],
            "e_in": nc.dram_tensor(f"ag_e_in{i}", [HS, B], F32),
            "e_out": nc.dram_tensor(f"ag_e_out{i}", [H, B], F32,
                                    addr_space="Shared"),
            "d_in": nc.dram_tensor(f"ag_d_in{i}", [HS, B], F32),
            "d_out": nc.dram_tensor(f"ag_d_out{i}", [H, B], F32,
                                    addr_space="Shared"),
        })

    RG = [list(range(NCORES))]

    def allgather(in_t, out_t, nrows):
        if cc_mode == "real":
            nc.gpsimd.collective_compute(
                "AllGather", mybir.AluOpType.bypass,
                replica_groups=RG,
                ins=[in_t.ap().opt()],
                outs=[out_t.ap().opt()])
        else:  # selfcopy: timing bisect only (numerics wrong)
            for r in range(NCORES):
                nc.sync.dma_start(
                    out_t.ap()[r * nrows:(r + 1) * nrows, :], in_t.ap())

    with tile.TileContext(nc) as tc:
        with tc.tile_pool(name="weights", bufs=1) as wp, \
             tc.tile_pool(name="state", bufs=1) as st, \
             tc.tile_pool(name="mmps", bufs=4, space="PSUM") as ps, \
             tc.tile_pool(name="trps", bufs=3, space="PSUM") as tr, \
             tc.tile_pool(name="epspool", bufs=2) as epp:

            # ---------------- resident weights ----------------
            w2_sb = wp.tile([128, KL * GS], BF16)    # 32KB/part
            _load_T(nc, w2_sb, w2t, KL, GS, 8)
            w3_sb = wp.tile([128, KH * GS], F32R)
            _load_T(nc, w3_sb, w3t, KH, GS, 2)
            whh_sb = wp.tile([128, KH * GS], F32R)
            _load_T(nc, whh_sb, whht, KH, GS, 2)
            wms_sb = wp.tile([128, KH * GS], F32R)
            _load_T(nc, wms_sb, wmst, KH, GS, 2)
            wid_sb = wp.tile([128, KZ * GS], F32R)
            _load_T(nc, wid_sb, widt, KZ, GS, 1)
            whd_sb = wp.tile([128, KH * GS], F32R)
            _load_T(nc, whd_sb, whdt, KH, GS, 2)
            wwr_sb = wp.tile([128, KH * LS], F32R)
            _load_T(nc, wwr_sb, wwrt, KH, LS, 2)

            be_sb = wp.tile([B, GS], F32)
            nc.sync.dma_start(be_sb[:], bias_e)
            bms_sb = None
            if not zb_ms:
                bms_sb = wp.tile([B, GS], F32)
                nc.sync.dma_start(bms_sb[:], bias_ms)
            bd_sb = None
            if not zb_d:
                bd_sb = wp.tile([B, GS], F32)
                nc.sync.dma_start(bd_sb[:], bias_d)
            bw_sb = None
            if not zb_w:
                bw_sb = wp.tile([B, LS], F32)
                nc.sync.dma_start(bw_sb[:], bias_w)

            ident = wp.tile([128, 128], F32)
            make_identity(nc, ident[:])

            # persistent state
            c_t = st.tile([B, LS], F32)
            c_enc = st.tile([B, HS], F32)
            c_dec = st.tile([B, HS], F32)
            heT = st.tile([128, KH * B], F32R)   # h_enc.T full (8 slots)
            hdT = st.tile([128, KH * B], F32R)
            sT = st.tile([128, KL * B], BF16)    # S.T full (32 slots)
            const_sb = st.tile([B, GS], F32)

            for rep in range(repeats):
                # -------------- init state --------------
                nc.sync.dma_start(c_t[:], c0b)
                nc.gpsimd.memset(c_enc[:], 0.0)
                nc.gpsimd.memset(c_dec[:], 0.0)
                _load_T(nc, heT, h0eT, KH, B, 2)
                _load_T(nc, hdT, h0dT, KH, B, 2)

                # ---- preamble: const = x@(W1+W2)[Gk].T + bias_e ----
                with tc.tile_pool(name="pre", bufs=3) as pre:
                    cst_ps = ps.tile([B, GS], F32, tag="mm")
                    for c in range(8):
                        xT_sb = pre.tile([128, 4 * B], F32R, tag="xT")
                        src = xT[c * 512:(c + 1) * 512, :]
                        nc.sync.dma_start(
                            xT_sb[:].rearrange("p (t n) -> p t n", t=4),
                            src.rearrange("(t p) n -> p t n", t=4))
                        w12_sb = pre.tile([128, 4 * GS], F32R, tag="w12")
                        src = w12t[c * 512:(c + 1) * 512, :]
                        nc.sync.dma_start(
                            w12_sb[:].rearrange("p (t n) -> p t n", t=4),
                            src.rearrange("(t p) n -> p t n", t=4))
                        for j in range(4):
                            kt = c * 4 + j
                            nc.tensor.matmul(
                                cst_ps[:],
                                xT_sb[:, j * B:(j + 1) * B],
                                w12_sb[:, j * GS:(j + 1) * GS],
                                start=(kt == 0), stop=(kt == KL - 1))
                    nc.vector.tensor_add(const_sb[:], cst_ps[:], be_sb[:])

                # -------------- time loop --------------
                with tc.tile_pool(name="work", bufs=2) as wk:
                    for t in range(T):
                        bufs = ag[t % 2]
                        eps_sb = epp.tile([B, Z], F32, tag="eps")
                        nc.sync.dma_start(eps_sb[:], eps_all[t])
                        # (1) enc-gate PSUM opens with Whh (h_encT[t-1] is
                        # available early) -> PE busy while AG(h_dec) lands.
                        ge_ps = ps.tile([B, GS], F32, tag="mm")
                        for kt in range(KH):
                            nc.tensor.matmul(
                                ge_ps[:], heT[:, kt * B:(kt + 1) * B],
                                whh_sb[:, kt * GS:(kt + 1) * GS],
                                start=(kt == 0), stop=False,
                                skip_group_check=True)

                        # (2) canvas update: c_t += h_dec@W_write.T + b_w
                        if t > 0:
                            cd_ps = ps.tile([B, LS], F32, tag="mm")
                            for kt in range(KH):
                                nc.tensor.matmul(
                                    cd_ps[:], hdT[:, kt * B:(kt + 1) * B],
                                    wwr_sb[:, kt * LS:(kt + 1) * LS],
                                    start=(kt == 0), stop=(kt == KH - 1),
                                    skip_group_check=True)
                            nc.vector.tensor_add(c_t[:], c_t[:], cd_ps[:])
                            if not zb_w:
                                nc.vector.tensor_add(c_t[:], c_t[:], bw_sb[:])

                        # (3) S = sigmoid(c_t); transpose; AG contribution
                        # t=0: S is known on the host (sigmoid of c_0) --
                        # preload the full S.T and skip the gather.
                        if t > 0:
                            s_b = wk.tile([B, LS], F32, tag="s_b")
                            nc.scalar.activation(s_b[:], c_t[:], AFT.Sigmoid)
                            s_ctb = wk.tile([128, 4 * B], BF16, tag="s_ctb")
                            for j in range(4):
                                tp = tr.tile([128, 128], F32, tag="tr")
                                nc.tensor.transpose(
                                    tp[:], s_b[:, j * 128:(j + 1) * 128],
                                    ident[:])
                                nc.vector.tensor_copy(
                                    s_ctb[:, j * B:(j + 1) * B], tp[:])
                            tph = 4 // s_split   # local tiles per split
                            for h in range(s_split):
                                nc.sync.dma_start(
                                    bufs["s_in"][h].ap()
                                    .rearrange("(t p) n -> p t n", t=tph),
                                    s_ctb[:, h * tph * B:(h + 1) * tph * B]
                                    .rearrange("p (t n) -> p t n", t=tph))
                                allgather(bufs["s_in"][h], bufs["s_out"][h],
                                          LS // s_split)

                        # (4) W3 term (h_decT): fills AG(S) latency
                        for kt in range(KH):
                            nc.tensor.matmul(
                                ge_ps[:], hdT[:, kt * B:(kt + 1) * B],
                                w3_sb[:, kt * GS:(kt + 1) * GS],
                                start=False, stop=False, skip_group_check=True)

                        # (5) load S.T full; -S@W2 accumulation (32 MMs)
                        if t == 0:
                            _load_T(nc, sT, s0T, KL, B, 8)
                        else:
                            for h in range(s_split):
                                for c in range(8):
                                    rows = (LS // s_split)
                                    src = bufs["s_out"][h].ap()[
                                        c * rows:(c + 1) * rows, :]
                                    slot = c * 4 + h * tph
                                    nc.sync.dma_start(
                                        sT[:, slot * B:(slot + tph) * B]
                                        .rearrange("p (t n) -> p t n", t=tph),
                                        src.rearrange("(t p) n -> p t n",
                                                      t=tph))
                        for kt in range(KL):
                            nc.tensor.matmul(
                                ge_ps[:], sT[:, kt * B:(kt + 1) * B],
                                w2_sb[:, kt * GS:(kt + 1) * GS],
                                start=False, stop=(kt == KL - 1),
                                skip_group_check=True)

                        # (6) encoder LSTM pointwise
                        ge = wk.tile([B, GS], F32, tag="ge")
                        nc.vector.tensor_add(ge[:], ge_ps[:], const_sb[:])
                        if debug and t == 0 and rep == 0:
                            nc.sync.dma_start(dbg["dbg_ge"], ge[:])
                        sif = wk.tile([B, 384], F32, tag="sif")
                        nc.scalar.activation(sif[:], ge[:, 0:384], AFT.Sigmoid)
                        tg = wk.tile([B, HS], F32, tag="tg")
                        nc.scalar.activation(tg[:], ge[:, 384:512], AFT.Tanh)
                        t2 = wk.tile([B, HS], F32, tag="t2")
                        nc.vector.tensor_mul(t2[:], sif[:, 0:128], tg[:])
                        nc.vector.tensor_mul(c_enc[:], c_enc[:],
                                             sif[:, 128:256])
                        nc.vector.tensor_add(c_enc[:], c_enc[:], t2[:])
                        tc_ = wk.tile([B, HS], F32, tag="tc_")
                        nc.scalar.activation(tc_[:], c_enc[:], AFT.Tanh)
                        h_e = wk.tile([B, HS], F32, tag="h_e")
                        nc.vector.tensor_mul(h_e[:], sif[:, 256:384], tc_[:])

                        # (7) transpose h_enc; AG
                        tp = tr.tile([128, 128], F32, tag="tr")
                        nc.tensor.transpose(tp[:], h_e[:], ident[:])
                        he_ctb = wk.tile([128, B], F32R, tag="he_ctb")
                        nc.vector.tensor_copy(he_ctb[:], tp[:])
                        nc.sync.dma_start(bufs["e_in"].ap(),
                                          he_ctb[:].bitcast(F32))
                        allgather(bufs["e_in"], bufs["e_out"], HS)

                        # (8) dec-gate PSUM opens with Whd (h_decT[t-1]):
                        # fills AG(h_enc) latency
                        gd_ps = ps.tile([B, GS], F32, tag="mm")
                        for kt in range(KH):
                            nc.tensor.matmul(
                                gd_ps[:], hdT[:, kt * B:(kt + 1) * B],
                                whd_sb[:, kt * GS:(kt + 1) * GS],
                                start=(kt == 0), stop=False,
                                skip_group_check=True)

                        # (9) reload h_encT full; musig
                        for c in range(2):
                            src = bufs["e_out"].ap()[c * 512:(c + 1) * 512, :]
                            nc.sync.dma_start(
                                heT[:, c * 4 * B:(c + 1) * 4 * B]
                                .rearrange("p (t n) -> p t n", t=4),
                                src.rearrange("(t p) n -> p t n", t=4)
                                .bitcast(F32R))
                        ms_ps = ps.tile([B, GS], F32, tag="mm")
                        for kt in range(KH):
                            nc.tensor.matmul(
                                ms_ps[:], heT[:, kt * B:(kt + 1) * B],
                                wms_sb[:, kt * GS:(kt + 1) * GS],
                                start=(kt == 0), stop=(kt == KH - 1),
                                skip_group_check=True)
                        if zb_ms and not debug:
                            ms = ms_ps
                        else:
                            ms = wk.tile([B, GS], F32, tag="ms")
                            if zb_ms:
                                nc.vector.tensor_copy(ms[:], ms_ps[:])
                            else:
                                nc.vector.tensor_add(ms[:], ms_ps[:],
                                                     bms_sb[:])
                            if debug and t == T - 1 and rep == 0:
                                nc.sync.dma_start(dbg["dbg_ms"], ms[:])

                        # (10) z = eps*exp(logsig) + mu; exp = sig(x)/sig(-x)
                        spz = wk.tile([B, Z], F32, tag="spz")
                        nc.scalar.activation(spz[:], ms[:, 256:512],
                                             AFT.Sigmoid)
                        snz = wk.tile([B, Z], F32, tag="snz")
                        nc.scalar.activation(snz[:], ms[:, 256:512],
                                             AFT.Sigmoid, scale=-1.0)
                        rnz = wk.tile([B, Z], F32, tag="rnz")
                        nc.vector.reciprocal(rnz[:], snz[:])
                        nc.vector.tensor_mul(rnz[:], spz[:], rnz[:])
                        nc.vector.tensor_mul(rnz[:], eps_sb[:], rnz[:])
                        z = wk.tile([B, Z], F32, tag="z")
                        nc.vector.tensor_add(z[:], rnz[:], ms[:, 0:256])

                        # (11) z.T (2 tiles); finish decoder gates with Wid
                        zT = wk.tile([128, KZ * B], F32R, tag="zT")
                        for j in range(KZ):
                            tp = tr.tile([128, 128], F32, tag="tr")
                            nc.tensor.transpose(
                                tp[:], z[:, j * 128:(j + 1) * 128], ident[:])
                            nc.vector.tensor_copy(
                                zT[:, j * B:(j + 1) * B], tp[:])
                        for kt in range(KZ):
                            nc.tensor.matmul(
                                gd_ps[:], zT[:, kt * B:(kt + 1) * B],
                                wid_sb[:, kt * GS:(kt + 1) * GS],
                                start=False, stop=(kt == KZ - 1),
                                skip_group_check=True)

                        # (12) decoder LSTM pointwise
                        if zb_d:
                            gd = gd_ps
                        else:
                            gd = wk.tile([B, GS], F32, tag="gd")
                            nc.vector.tensor_add(gd[:], gd_ps[:], bd_sb[:])
                        sifd = wk.tile([B, 384], F32, tag="sifd")
                        nc.scalar.activation(sifd[:], gd[:, 0:384],
                                             AFT.Sigmoid)
                        tgd = wk.tile([B, HS], F32, tag="tgd")
                        nc.scalar.activation(tgd[:], gd[:, 384:512], AFT.Tanh)
                        t2d = wk.tile([B, HS], F32, tag="t2d")
                        nc.vector.tensor_mul(t2d[:], sifd[:, 0:128], tgd[:])
                        nc.vector.tensor_mul(c_dec[:], c_dec[:],
                                             sifd[:, 128:256])
                        nc.vector.tensor_add(c_dec[:], c_dec[:], t2d[:])
                        tcd = wk.tile([B, HS], F32, tag="tcd")
                        nc.scalar.activation(tcd[:], c_dec[:], AFT.Tanh)
                        h_d = wk.tile([B, HS], F32, tag="h_d")
                        nc.vector.tensor_mul(h_d[:], sifd[:, 256:384], tcd[:])

                        # (13) transpose h_dec; AG; reload h_decT full
                        tp = tr.tile([128, 128], F32, tag="tr")
                        nc.tensor.transpose(tp[:], h_d[:], ident[:])
                        hd_ctb = wk.tile([128, B], F32R, tag="hd_ctb")
                        nc.vector.tensor_copy(hd_ctb[:], tp[:])
                        nc.sync.dma_start(bufs["d_in"].ap(),
                                          hd_ctb[:].bitcast(F32))
                        allgather(bufs["d_in"], bufs["d_out"], HS)
                        for c in range(2):
                            src = bufs["d_out"].ap()[c * 512:(c + 1) * 512, :]
                            nc.sync.dma_start(
                                hdT[:, c * 4 * B:(c + 1) * 4 * B]
                                .rearrange("p (t n) -> p t n", t=4),
                                src.rearrange("(t p) n -> p t n", t=4)
                                .bitcast(F32R))

                    # -------------- epilogue --------------
                    cd_ps = ps.tile([B, LS], F32, tag="mm")
                    for kt in range(KH):
                        nc.tensor.matmul(
                            cd_ps[:], hdT[:, kt * B:(kt + 1) * B],
                            wwr_sb[:, kt * LS:(kt + 1) * LS],
                            start=(kt == 0), stop=(kt == KH - 1),
                            skip_group_check=True)
                    nc.vector.tensor_add(c_t[:], c_t[:], cd_ps[:])
                    if not zb_w:
                        nc.vector.tensor_add(c_t[:], c_t[:], bw_sb[:])
                    if debug and rep == 0:
                        nc.sync.dma_start(dbg["dbg_ct"], c_t[:])
                        nc.sync.dma_start(
                            dbg["dbg_heT"].rearrange("(t p) n -> p t n", t=KH),
                            heT[:].rearrange("p (t n) -> p t n", t=KH)
                            .bitcast(F32))
                        nc.sync.dma_start(
                            dbg["dbg_hdT"].rearrange("(t p) n -> p t n", t=KH),
                            hdT[:].rearrange("p (t n) -> p t n", t=KH)
                            .bitcast(F32))
                    o_sb = wk.tile([B, LS], F32, tag="s_b")
                    nc.scalar.activation(o_sb[:], c_t[:], AFT.Sigmoid)
                    nc.sync.dma_start(out, o_sb[:])

    nc.compile()
    return nc


def _prep_inputs(inputs):
    """Build the 8 per-core input maps from the full problem inputs."""
    f = np.float32
    bf = ml_dtypes.bfloat16
    x = np.asarray(inputs["x"], f)
    eps = np.ascontiguousarray(np.asarray(inputs["eps"], f))
    Wie = np.asarray(inputs["W_ih_enc"], f)
    Whe = np.asarray(inputs["W_hh_enc"], f)
    Wms = np.concatenate([np.asarray(inputs["W_mu"], f),
                          np.asarray(inputs["W_sigma"], f)], 0)
    bms = np.concatenate([np.asarray(inputs["b_mu"], f),
                          np.asarray(inputs["b_sigma"], f)], 0)
    Wid = np.asarray(inputs["W_ih_dec"], f)
    Whd = np.asarray(inputs["W_hh_dec"], f)
    Wwr = np.asarray(inputs["W_write"], f)
    be = np.asarray(inputs["b_ih_enc"], f) + np.asarray(inputs["b_hh_enc"], f)
    bd = np.asarray(inputs["b_ih_dec"], f) + np.asarray(inputs["b_hh_dec"], f)
    bw = np.asarray(inputs["b_write"], f)
    c0 = np.asarray(inputs["c_0"], f)
    h0e = np.asarray(inputs["h_0_enc"], f)
    h0d = np.asarray(inputs["h_0_dec"], f)

    xT = np.ascontiguousarray(x.T)
    h0eT = np.ascontiguousarray(np.repeat(h0e[:, None], B, 1))
    h0dT = np.ascontiguousarray(np.repeat(h0d[:, None], B, 1))
    wmst = np.ascontiguousarray(Wms.T)
    bms_b = np.ascontiguousarray(np.broadcast_to(bms, (B, GS)))

    s0 = 1.0 / (1.0 + np.exp(-c0[0].astype(np.float64)))
    s0T = np.ascontiguousarray(
        np.repeat(s0[:, None].astype(f), B, 1)).astype(bf)

    in_maps = []
    for k in range(NCORES):
        # gate row order [i, f, o, g] so sigmoid covers one contiguous block
        Gk = np.concatenate(
            [np.arange(HS * k, HS * (k + 1)) + j * H for j in (0, 1, 3, 2)])
        Lk = slice(LS * k, LS * (k + 1))
        W1g = Wie[Gk, :L]
        W2g = Wie[Gk, L:2 * L]
        in_maps.append({
            "xT": xT,
            "w12t": np.ascontiguousarray((W1g + W2g).T),
            "w2t": np.ascontiguousarray(-W2g.T).astype(bf),
            "w3t": np.ascontiguousarray(Wie[Gk, 2 * L:].T),
            "whht": np.ascontiguousarray(Whe[Gk].T),
            "wmst": wmst,
            "widt": np.ascontiguousarray(Wid[Gk].T),
            "whdt": np.ascontiguousarray(Whd[Gk].T),
            "wwrt": np.ascontiguousarray(Wwr[Lk].T),
            "bias_e": np.ascontiguousarray(np.broadcast_to(be[Gk], (B, GS))),
            "bias_ms": bms_b,
            "bias_d": np.ascontiguousarray(np.broadcast_to(bd[Gk], (B, GS))),
            "bias_w": np.ascontiguousarray(np.broadcast_to(bw[Lk], (B, LS))),
            "c0b": np.ascontiguousarray(np.broadcast_to(c0[0, Lk], (B, LS))),
            "h0eT": h0eT,
            "h0dT": h0dT,
            "eps": eps,
            "s0T": s0T,
        })
    zb = {"zb_ms": not np.any(bms), "zb_d": not np.any(bd),
          "zb_w": not np.any(bw)}
    return in_maps, zb


_NC_CACHE = {}


def _get_nc(repeats=1, debug=False, cc_mode="real", s_split=4, **zb):
    key = (repeats, debug, cc_mode, s_split, tuple(sorted(zb.items())))
    if key not in _NC_CACHE:
        _NC_CACHE[key] = build(repeats=repeats, debug=debug, cc_mode=cc_mode,
                               s_split=s_split, **zb)
    return _NC_CACHE[key]


def run(inputs, repeats=1, debug=False, cc_mode="real", s_split=4):
    in_maps, zb = _prep_inputs(inputs)
    nc = _get_nc(repeats=repeats, debug=debug, cc_mode=cc_mode,
                 s_split=s_split, **zb)
    res = run_bass_kernel_spmd(nc, in_maps, core_ids=list(range(NCORES)))
    full = np.concatenate([res.results[k]["out"] for k in range(NCORES)], 1)
    return full.astype(np.float32), res


def kernel(**inputs) -> np.ndarray:
    out, _ = run(inputs, repeats=1, debug=False)
    return out
